# revision 1
# baseline (speedup 1.0000x reference)
import numpy as np
import jax
import jax.numpy as jnp

# GCNConv with dense adjacency, B=8, N=2048, F_IN=F_OUT=256.
# Data-parallel: batch dim B=8 sharded 1-per-core across the 8 NeuronCores,
# W and b replicated (closed over as constants).
#
# Math (avoids materializing A_hat or adj_norm [N,N]):
#   A_hat = A + I;  deg = A_hat.sum(-1) = A.sum(-1) + 1;  d = deg^{-1/2}
#   out = d * (A_hat @ (d * (x @ W))) + b
#       = d * (A @ h2 + h2) + b   where h2 = d[:,None] * (x @ W)

_B = 8


def _per_core(x, adj, W, b):
    deg = jnp.sum(adj, axis=-1) + 1.0                # [N]
    d = deg ** -0.5
    d = jnp.where(jnp.isinf(d), 0.0, d)
    h = x @ W                                        # [N, F_OUT]
    h2 = d[:, None] * h
    tmp = adj @ h2 + h2                              # A_hat @ h2 without A+I
    return d[:, None] * tmp + b


def kernel(x, adj, W, b):
    devs = jax.devices()[:_B]
    f = jax.pmap(_per_core, in_axes=(0, 0, None, None), devices=devs)
    out = f(jnp.asarray(x), jnp.asarray(adj), jnp.asarray(W), jnp.asarray(b))
    return np.asarray(out, dtype=np.float32)
